# revision 6
# baseline (speedup 1.0000x reference)
"""Deformable Conv2d (3x3, stride 1, pad 1) on 8 Trainium2 NeuronCores.

Data-parallel over batch: core b handles sample b.

Per-core pipeline (channel-major layout, C=128 on partitions):
  1. x -> zero-padded x_pad [128, 100*100+pad] fp32 (orig (y,x) at (y+2)*100+(x+2))
  2. 4-corner texture V [128, 10000, 4] bf16: V[:, j, m] = x_pad[j + {0,1,100,101}[m]]
  3. offset conv via 9 accumulating matmuls; stationary weights packed so the
     18 offset channels are replicated in all four 32-partition quadrants
     (enables stream_shuffle broadcast later)
  4. DVE pipeline: p2 = off + grid + 2 (clamped), floor/frac split,
     flat corner index = 100*iy + ix (int16), frac tensor wY bf16
  5. per tap: wrapped idx layout for ap_gather (8 small DMAs)
  6. per (chunk, tap): stream_shuffle-broadcast bilinear weights, ap_gather
     4 corners, weighted-sum on DVE, accumulate taps into PSUM via matmul
     with conv_w, add bias, DMA out.
"""
import numpy as np
import ml_dtypes
from contextlib import ExitStack

import concourse.bass as bass
import concourse.bacc as bacc
import concourse.tile as tile
import concourse.mybir as mybir
from concourse.bass_utils import run_bass_kernel_spmd

F32 = mybir.dt.float32
BF16 = mybir.dt.bfloat16
I16 = mybir.dt.int16
I32 = mybir.dt.int32

B, C, H, W, O = 8, 128, 96, 96, 128
K = 3
K2 = 9
N = H * W              # 9216 positions
PW = 100               # padded width/height
NPOS = PW * PW         # 10000
XPAD = NPOS + 104      # over-alloc so V-build shifted reads stay in bounds
NCHUNK = 6
CH = N // NCHUNK       # 1536 positions per chunk
ROWT = 24              # offset-conv tiles (4 rows x 96 cols = 384)
CLAMP_HI = 96.996 + 2.0  # clamp on p2 = py + 2

AG = mybir.AluOpType

_CACHE = {}


def _build():
    nc = bacc.Bacc("TRN2", target_bir_lowering=False, debug=False, num_devices=8)
    x_in = nc.dram_tensor("x", [C, N], F32, kind="ExternalInput").ap()
    low_in = nc.dram_tensor("low", [C, K2 * 128], F32, kind="ExternalInput").ap()
    ob_in = nc.dram_tensor("ob", [128, 1], F32, kind="ExternalInput").ap()
    ww_in = nc.dram_tensor("ww", [C, K2 * 128], F32, kind="ExternalInput").ap()
    cb_in = nc.dram_tensor("cb", [128, 1], F32, kind="ExternalInput").ap()
    grid_in = nc.dram_tensor("grid", [128, N], F32, kind="ExternalInput").ap()
    out_d = nc.dram_tensor("out", [128, N], F32, kind="ExternalOutput").ap()

    PCH = 384  # pipeline chunk

    with tile.TileContext(nc) as tc, ExitStack() as ctx:
        persist = ctx.enter_context(tc.tile_pool(name="persist", bufs=1))
        V = persist.tile([128, 4 * NPOS], BF16)
        V3 = V[:].rearrange("p (n d) -> p n d", d=4)
        wY = persist.tile([128, N], BF16)
        flat16 = persist.tile([128, N], I16)
        idxw = persist.tile([128, K2 * 576], I16)
        ww = persist.tile([128, K2 * 128], F32)
        nc.sync.dma_start(ww[:], ww_in[:])
        cbp = persist.tile([128, 1], F32)
        nc.sync.dma_start(cbp[:], cb_in[:])

        with tc.tile_pool(name="pool1", bufs=1) as pool1:
            # --- load x into padded buffer ---
            x_pad = pool1.tile([128, XPAD], F32)
            nc.vector.memset(x_pad[:], 0.0)
            nc.sync.dma_start(
                bass.AP(x_pad.tensor, x_pad.offset + 2 * PW + 2,
                        [[XPAD, 128], [PW, H], [1, W]]),
                x_in[:].rearrange("c (h w) -> c h w", h=H))
            low = pool1.tile([128, K2 * 128], F32)
            nc.sync.dma_start(low[:], low_in[:])
            obp = pool1.tile([128, 1], F32)
            nc.sync.dma_start(obp[:], ob_in[:])

            # --- 4-corner texture V (bf16) ---
            for m, dlt in enumerate((0, 1, PW, PW + 1)):
                nc.scalar.copy(
                    V3[:, :, m],
                    bass.AP(x_pad.tensor, x_pad.offset + dlt,
                            [[XPAD, 128], [1, NPOS]]))

            # --- offset conv (quadrant-replicated channels) ---
            offs = pool1.tile([128, N], BF16)
            with tc.tile_pool(name="ps_off", bufs=2, space="PSUM") as ps_off:
                for t in range(ROWT):
                    ps = ps_off.tile([128, 384], F32)
                    for a in range(K):
                        for b in range(K):
                            kk = a * K + b
                            rhs = bass.AP(
                                x_pad.tensor,
                                x_pad.offset + (4 * t + a) * PW + b + PW + 1,
                                [[XPAD, 128], [PW, 4], [1, W]])
                            nc.tensor.matmul(
                                ps[:], low[:, kk * 128:(kk + 1) * 128], rhs,
                                start=(kk == 0), stop=(kk == 8))
                    nc.vector.tensor_scalar(
                        offs[:, t * 384:(t + 1) * 384], ps[:], obp[:], 0.0,
                        op0=AG.add, op1=AG.add)

            # --- index/weight pipeline ---
            mask_xe = [min(i + 1, 31) if i % 2 == 0 else i for i in range(32)]
            with tc.tile_pool(name="pipe", bufs=1) as pipe:
                for cchunk in range(N // PCH):
                    sl = slice(cchunk * PCH, (cchunk + 1) * PCH)
                    g = pipe.tile([128, PCH], F32, tag="g")
                    nc.sync.dma_start(g[:], grid_in[:, sl])
                    t0 = pipe.tile([128, PCH], F32, tag="t0")
                    nc.vector.tensor_add(t0[:], offs[:, sl], g[:])
                    t1 = pipe.tile([128, PCH], F32, tag="t1")
                    nc.vector.tensor_scalar(t1[:], t0[:], CLAMP_HI, 0.0,
                                            op0=AG.min, op1=AG.max)
                    i0 = pipe.tile([128, PCH], I32, tag="i0")
                    nc.vector.tensor_copy(i0[:], t1[:])
                    f0 = pipe.tile([128, PCH], F32, tag="f0")
                    nc.vector.tensor_copy(f0[:], i0[:])
                    gt = pipe.tile([128, PCH], F32, tag="gt")
                    nc.vector.tensor_tensor(gt[:], f0[:], t1[:], op=AG.is_gt)
                    fl = pipe.tile([128, PCH], F32, tag="fl")
                    nc.vector.tensor_sub(fl[:], f0[:], gt[:])
                    nc.vector.tensor_sub(wY[:, sl], t1[:], fl[:])
                    fx = pipe.tile([128, PCH], F32, tag="fx")
                    nc.vector.stream_shuffle(fx[:], fl[:], mask_xe)
                    ff = pipe.tile([128, PCH], F32, tag="ff")
                    nc.vector.scalar_tensor_tensor(
                        ff[:], fl[:], 100.0, fx[:], op0=AG.mult, op1=AG.add)
                    nc.vector.tensor_copy(flat16[:, sl], ff[:])

        # --- wrapped idx layout: idxw[16g+r, k*576+f] = flat16[2k, 16f+r] ---
        # bounce through DRAM scratch (free-form APs) to cross partitions
        dscr = nc.dram_tensor("idx_scratch", [K2, N], I16, kind="Internal")
        for k in range(K2):
            nc.sync.dma_start(
                bass.AP(dscr, k * N, [[N, 1], [1, N]]),
                flat16[2 * k:2 * k + 1, :])
        for k in range(K2):
            src = bass.AP(dscr, k * N, [[1, 16], [16, 576]])
            for gq in range(8):
                nc.sync.dma_start(
                    idxw[16 * gq:16 * (gq + 1), k * 576:(k + 1) * 576], src)

        # --- main loop: chunks x taps ---
        with tc.tile_pool(name="gpool", bufs=2) as gpool, \
             tc.tile_pool(name="work", bufs=1) as work, \
             tc.tile_pool(name="outp", bufs=1) as outp, \
             tc.tile_pool(name="ps_main", bufs=2, space="PSUM") as ps_main:
            for cchunk in range(NCHUNK):
                sl = slice(cchunk * CH, (cchunk + 1) * CH)
                ps = ps_main.tile([128, CH], F32)
                for k in range(K2):
                    wyb = work.tile([128, CH], BF16, tag="wyb")
                    nc.vector.stream_shuffle(wyb[:], wY[:, sl], [2 * k] * 32)
                    wxb = work.tile([128, CH], BF16, tag="wxb")
                    nc.vector.stream_shuffle(wxb[:], wY[:, sl], [2 * k + 1] * 32)
                    G = gpool.tile([128, CH * 4], BF16, tag="G")
                    G3 = G[:].rearrange("p (n d) -> p n d", d=4)
                    nc.gpsimd.ap_gather(
                        G3, V3,
                        idxw[:, k * 576 + 96 * cchunk: k * 576 + 96 * (cchunk + 1)],
                        channels=128, num_elems=NPOS, d=4, num_idxs=CH)
                    uy = work.tile([128, CH], F32, tag="uy")
                    nc.vector.tensor_scalar(uy[:], wyb[:], -1.0, 1.0,
                                            op0=AG.mult, op1=AG.add)
                    ux = work.tile([128, CH], F32, tag="ux")
                    nc.vector.tensor_scalar(ux[:], wxb[:], -1.0, 1.0,
                                            op0=AG.mult, op1=AG.add)
                    S = work.tile([128, CH], F32, tag="S")
                    for m, (wa, wb_) in enumerate(((uy, ux), (uy, wxb),
                                                   (wyb, ux), (wyb, wxb))):
                        p = work.tile([128, CH], F32, tag="p")
                        nc.vector.tensor_mul(p[:], wa[:], wb_[:])
                        if m == 0:
                            nc.vector.tensor_mul(S[:], p[:], G3[:, :, m])
                        else:
                            mm = work.tile([128, CH], F32, tag="mm")
                            nc.vector.tensor_mul(mm[:], p[:], G3[:, :, m])
                            nc.vector.tensor_add(S[:], S[:], mm[:])
                    for j in range(CH // 512):
                        nc.tensor.matmul(
                            ps[:, 512 * j:512 * (j + 1)],
                            ww[:, k * 128:(k + 1) * 128],
                            S[:, 512 * j:512 * (j + 1)],
                            start=(k == 0), stop=(k == 8))
                ob = outp.tile([128, CH], F32, tag="ob")
                nc.vector.tensor_scalar(ob[:], ps[:], cbp[:], 0.0,
                                        op0=AG.add, op1=AG.add)
                nc.sync.dma_start(out_d[:, sl], ob[:])
    nc.compile()
    return nc


def _pack_inputs(x, offset_w, offset_b, conv_w, conv_b):
    """Host-side packing -> per-core input maps."""
    x = np.asarray(x, np.float32)
    offset_w = np.asarray(offset_w, np.float32)
    offset_b = np.asarray(offset_b, np.float32)
    conv_w = np.asarray(conv_w, np.float32)
    conv_b = np.asarray(conv_b, np.float32)

    # offset conv stationary: low[c, 32q+ch] = offset_w[ch, c, a, b] per tap
    low = np.zeros((C, K2, 128), np.float32)
    for q in range(4):
        low[:, :, 32 * q:32 * q + 18] = offset_w.reshape(18, C, K2).transpose(1, 2, 0)
    low = low.reshape(C, K2 * 128)
    ob = np.zeros((128, 1), np.float32)
    for q in range(4):
        ob[32 * q:32 * q + 18, 0] = offset_b
    ww = conv_w.reshape(O, C, K2).transpose(1, 2, 0).reshape(C, K2 * 128).copy()
    cb = conv_b.reshape(128, 1).copy()

    # grid const: lane 2k: y + 1 + ky + 2 ; lane 2k+1: x + 1 + kx + 2
    yy, xx = np.meshgrid(np.arange(H), np.arange(W), indexing="ij")
    grid = np.zeros((128, N), np.float32)
    for q in range(4):
        for k in range(K2):
            ky, kx = k // 3, k % 3
            grid[32 * q + 2 * k] = (yy.reshape(-1) + 1 + ky).astype(np.float32)
            grid[32 * q + 2 * k + 1] = (xx.reshape(-1) + 1 + kx).astype(np.float32)
    # p2 = off + (orig + 2): py = (y-1) + ky + off -> p2 = y + 1 + ky + off
    shared = {"low": low, "ob": ob, "ww": ww, "cb": cb, "grid": grid}
    in_maps = []
    for b in range(B):
        m = dict(shared)
        m["x"] = x[b].reshape(C, N).copy()
        in_maps.append(m)
    return in_maps


def kernel(x, offset_w, offset_b, conv_w, conv_b):
    if "nc" not in _CACHE:
        _CACHE["nc"] = _build()
    nc = _CACHE["nc"]
    in_maps = _pack_inputs(x, offset_w, offset_b, conv_w, conv_b)
    res = run_bass_kernel_spmd(nc, in_maps, core_ids=list(range(8)))
    out = np.stack([res.results[b]["out"].reshape(O, H, W) for b in range(B)])
    return out.astype(np.float32)


if __name__ == "__main__":
    rng = np.random.default_rng(0)
    x = rng.standard_normal((B, C, H, W)).astype(np.float32)
    ow = (rng.standard_normal((18, C, K, K)) * 0.01).astype(np.float32)
    ob_ = (rng.standard_normal(18) * 0.01).astype(np.float32)
    cw = (rng.standard_normal((O, C, K, K)) / np.sqrt(C * 9)).astype(np.float32)
    cb_ = (rng.standard_normal(O) * 0.01).astype(np.float32)
    y = kernel(x, ow, ob_, cw, cb_)
    print("out", y.shape, y.dtype, float(np.abs(y).max()))
